# revision 25
# baseline (speedup 1.0000x reference)
"""Trainium2 Bass kernel for nn_FFM (FFM layer: pre-matmul + gated complex-decay
scan + mix-matmul + LayerNorm + gated residual).

Sharding: data-parallel over B=8 across the 8 NeuronCores (one batch element
per core). Weights/tables replicated.

Per-core algorithm (T=2048, IN=1024, OUT=1024, M=C=32):
  pre = x @ pre_w (+pre_b)  ->  y|thru|gate ; gate=sigmoid
  u = y * in_gate
  Demodulated scan (w_t = decay_m * w_{t-1} + e^{-i b_c t} u_t) computed as two
  REAL first-order scans via the DVE tensor_tensor_scan instruction, with
  (m,c) on partitions and t on the free axis; s_t = e^{+i b_c t} * w_t.
  z = interleave(re,im)(s) @ mix_w (+mix_b);  LayerNorm(z);
  out = z*out_gate + thru*(1-out_gate)
"""

import numpy as np

import concourse.bass as bass
import concourse.tile as tile
from concourse import bacc, mybir
from concourse.bass_utils import run_bass_kernel_spmd

B, T, IN, OUT, M, C = 8, 2048, 1024, 1024, 32, 32
J = 2 * M + 2 * OUT  # 2112
MC = M * C  # 1024
EPS = 1e-5
TB = 512  # t-block for scan/mix phase
NBLK = T // TB  # 4
NTT = T // 128  # 16 t-tiles
F32 = mybir.dt.float32
F32R = mybir.dt.float32r
COPY = mybir.ActivationFunctionType.Copy

_compiled = None


def _build(pre_b_zero: bool, mix_b_zero: bool, srsi_bufs: int = 2):
    nc = bacc.Bacc("TRN2", target_bir_lowering=False, debug=False, num_devices=8)

    # ---- DRAM I/O ----
    xT_d = nc.dram_tensor("xT", [IN, T], F32R, kind="ExternalInput")
    pre_w_d = nc.dram_tensor("pre_w", [IN, J], F32R, kind="ExternalInput")
    pre_b_d = nc.dram_tensor("pre_b", [J], F32, kind="ExternalInput")
    mix_w_d = nc.dram_tensor("mix_w", [2 * MC, OUT], F32R, kind="ExternalInput")
    mix_b_d = nc.dram_tensor("mix_b", [OUT], F32, kind="ExternalInput")
    cos_d = nc.dram_tensor("cosT", [128, T], F32, kind="ExternalInput")
    sinn_d = nc.dram_tensor("sinnT", [128, T], F32, kind="ExternalInput")
    dec_d = nc.dram_tensor("decay_col", [128, 8], F32, kind="ExternalInput")
    identr_d = nc.dram_tensor("ident_r", [128, 128], F32R, kind="ExternalInput")
    replw_d = nc.dram_tensor("repl_w", [8, 32, 128], F32R, kind="ExternalInput")
    out_d = nc.dram_tensor("out", [T, OUT], F32, kind="ExternalOutput")
    s_d = nc.dram_tensor("s_out", [T, 2 * MC], F32, kind="ExternalOutput")
    # scratch between phases
    thru_d = nc.dram_tensor("thru_scr", [T, OUT], F32)
    og_d = nc.dram_tensor("og_scr", [T, OUT], F32)

    with tile.TileContext(nc) as tc:
        with (
            tc.tile_pool(name="consts", bufs=1) as consts,
            tc.tile_pool(name="mixw", bufs=1) as mixw_pool,
        ):
            mix_w_sb = mixw_pool.tile([128, 8, 2, OUT], F32R)
            nc.sync.dma_start(
                out=mix_w_sb,
                in_=mix_w_d.ap().rearrange("(q p two) o -> p q two o", p=128, two=2),
            )
            dec = consts.tile([128, 8], F32)
            ident_r = consts.tile([128, 128], F32R)
            nc.sync.dma_start(out=ident_r, in_=identr_d[:, :])
            nc.sync.dma_start(out=dec, in_=dec_d[:, :])
            uT = consts.tile([32, T], F32R)  # u transposed, filled in phase P
            repl_w = consts.tile([32, 8, 128], F32R)
            nc.sync.dma_start(
                out=repl_w, in_=replw_d.ap().rearrange("q k p -> k q p")
            )
            if not pre_b_zero:
                preb_bc = consts.tile([128, J], F32)
                nc.gpsimd.dma_start(
                    out=preb_bc,
                    in_=bass.AP(tensor=pre_b_d, offset=0, ap=[[0, 128], [1, J]]),
                )
            if not mix_b_zero:
                mixb_bc = consts.tile([128, OUT], F32)
                nc.gpsimd.dma_start(
                    out=mixb_bc,
                    in_=bass.AP(tensor=mix_b_d, offset=0, ap=[[0, 128], [1, OUT]]),
                )
            eps_t = consts.tile([128, 1], F32)
            nc.vector.memset(eps_t, EPS)

            # ============ PHASE P: pre matmul + gates ============
            with (
                tc.tile_pool(name="prew", bufs=1) as prew_pool,
                tc.tile_pool(name="pin", bufs=4) as pin,
                tc.tile_pool(name="pwork", bufs=2) as pwork,
                tc.tile_pool(name="pps", bufs=2, space="PSUM") as pps,
                tc.tile_pool(name="ppre", bufs=3, space="PSUM") as ppre,
            ):
                pre_w_sb = prew_pool.tile([128, 8, J], F32R)
                nc.sync.dma_start(
                    out=pre_w_sb,
                    in_=pre_w_d.ap().rearrange("(k p) j -> p k j", p=128),
                )
                JCS = [(0, 512), (512, 512), (1024, 512), (1536, 512), (2048, 64)]
                for tt in range(NTT):
                    xT = pin.tile([128, 8, 128], F32R, tag="xT")
                    nc.sync.dma_start(
                        out=xT,
                        in_=xT_d[:, tt * 128 : (tt + 1) * 128].rearrange(
                            "(k p) t -> p k t", p=128
                        ),
                    )
                    pre_t = pwork.tile([128, J], F32, tag="pret")
                    for j0, jn in JCS:
                        pre_ps = ppre.tile([128, 512], F32, tag="preps")
                        for k in range(8):
                            nc.tensor.matmul(
                                pre_ps[:, 0:jn],
                                xT[:, k, :],
                                pre_w_sb[:, k, j0 : j0 + jn],
                                start=(k == 0),
                                stop=(k == 7),
                            )
                        nc.scalar.activation(
                            out=pre_t[:, j0 : j0 + jn], in_=pre_ps[:, 0:jn], func=COPY
                        )
                    if not pre_b_zero:
                        nc.vector.tensor_add(pre_t, pre_t, preb_bc)
                    # gates
                    g_t = pwork.tile([128, M + OUT], F32, tag="gt")
                    nc.scalar.activation(
                        out=g_t,
                        in_=pre_t[:, M + OUT :],
                        func=mybir.ActivationFunctionType.Sigmoid,
                    )
                    u_t = pwork.tile([128, M], F32R, tag="ut")
                    nc.vector.tensor_mul(u_t, pre_t[:, 0:M], g_t[:, 0:M])
                    # transpose u -> uT[:, tt*128:]
                    up = pps.tile([32, 128], F32R, tag="xtp")
                    nc.tensor.transpose(up, u_t, ident_r)
                    nc.scalar.activation(
                        out=uT[:, tt * 128 : (tt + 1) * 128], in_=up, func=COPY
                    )
                    # spill thru / out_gate
                    nc.sync.dma_start(
                        out=thru_d[tt * 128 : (tt + 1) * 128, :],
                        in_=pre_t[:, M : M + OUT],
                    )
                    nc.sync.dma_start(
                        out=og_d[tt * 128 : (tt + 1) * 128, :], in_=g_t[:, M:]
                    )

            # ============ PHASE S: scan + mix + LN + gating ============
            with (
                tc.tile_pool(name="tabs", bufs=2) as tabs,
                tc.tile_pool(name="srsi", bufs=srsi_bufs) as srsi_pool,
                tc.tile_pool(name="swork", bufs=2) as swork,
                tc.tile_pool(name="wcar", bufs=2) as wcar,
                tc.tile_pool(name="sstage", bufs=2) as sstage,
                tc.tile_pool(name="zps", bufs=2, space="PSUM") as zps,
                tc.tile_pool(name="sps", bufs=2, space="PSUM") as sps,
            ):
                wr_prev = [None] * 8
                wi_prev = [None] * 8
                for blk in range(NBLK):
                    t0 = blk * TB
                    cos_b = tabs.tile([128, TB], F32, tag="cosb")
                    sinn_b = tabs.tile([128, TB], F32, tag="sinnb")
                    nc.sync.dma_start(out=cos_b, in_=cos_d[:, t0 : t0 + TB])
                    nc.sync.dma_start(out=sinn_b, in_=sinn_d[:, t0 : t0 + TB])
                    srsi = []
                    for q in range(8):
                        repl = sps.tile([128, TB], F32, tag="repl")
                        nc.tensor.matmul(
                            repl,
                            repl_w[:, q, :],
                            uT[:, t0 : t0 + TB],
                            start=True,
                            stop=True,
                        )
                        uc = swork.tile([128, TB], F32, tag="uc")
                        us = swork.tile([128, TB], F32, tag="us")
                        nc.vector.tensor_mul(uc, repl, cos_b)
                        nc.vector.tensor_mul(us, repl, sinn_b)
                        wr = swork.tile([128, TB], F32, tag="wr")
                        wi = swork.tile([128, TB], F32, tag="wi")
                        dq = dec[:, q : q + 1].to_broadcast((128, TB))
                        nc.vector.tensor_tensor_scan(
                            out=wr,
                            data0=dq,
                            data1=uc,
                            initial=0.0 if blk == 0 else wr_prev[q][:, 0:1],
                            op0=mybir.AluOpType.mult,
                            op1=mybir.AluOpType.add,
                        )
                        nc.vector.tensor_tensor_scan(
                            out=wi,
                            data0=dq,
                            data1=us,
                            initial=0.0 if blk == 0 else wi_prev[q][:, 0:1],
                            op0=mybir.AluOpType.mult,
                            op1=mybir.AluOpType.add,
                        )
                        warma = sps.tile([64, 64], F32, tag="repl")
                        nc.tensor.matmul(
                            warma, wi[:, 0:64], cos_b[:, 0:64], start=True, stop=True
                        )
                        if blk < NBLK - 1:
                            wrc = wcar.tile([128, 1], F32, tag=f"wrc{q}")
                            wic = wcar.tile([128, 1], F32, tag=f"wic{q}")
                            nc.scalar.activation(
                                out=wrc, in_=wr[:, TB - 1 : TB], func=COPY
                            )
                            nc.scalar.activation(
                                out=wic, in_=wi[:, TB - 1 : TB], func=COPY
                            )
                            wr_prev[q], wi_prev[q] = wrc, wic
                        # remodulate: sr = cos*wr + sinn*wi ; si = cos*wi - sinn*wr
                        m4 = swork.tile([128, TB], F32, tag="m4")
                        m2 = swork.tile([128, TB], F32, tag="m2")
                        sr = srsi_pool.tile([128, TB], F32R, tag=f"sr{q}")
                        si = srsi_pool.tile([128, TB], F32R, tag=f"si{q}")
                        nc.gpsimd.tensor_mul(m4, sinn_b, wi)
                        nc.vector.tensor_mul(m2, sinn_b, wr)
                        nc.vector.tensor_mul(uc, cos_b, wr)
                        nc.gpsimd.tensor_mul(us, cos_b, wi)
                        nc.vector.tensor_add(sr, uc, m4)
                        nc.gpsimd.tensor_sub(si, us, m2)
                        srsi.append((sr, si))
                        warmb = sps.tile([64, 64], F32, tag="repl")
                        nc.tensor.matmul(
                            warmb, sr[:, 0:64], ident_r[:, 0:64], start=True, stop=True
                        )
                    # ---- mix matmul + LN + gating + s-out per t-sub ----
                    for ts in range(TB // 128):
                        tg = t0 + ts * 128
                        z_ps = zps.tile([128, OUT], F32, tag="zps")
                        for kt in range(16):
                            qq, comp = kt // 2, kt % 2
                            lhs = srsi[qq][comp][:, ts * 128 : (ts + 1) * 128]
                            for oc in range(2):
                                nc.tensor.matmul(
                                    z_ps[:, oc * 512 : (oc + 1) * 512],
                                    lhs,
                                    mix_w_sb[:, qq, comp, oc * 512 : (oc + 1) * 512],
                                    start=(kt == 0),
                                    stop=(kt == 15),
                                )
                        z_t = swork.tile([128, OUT], F32, tag="zt")
                        nc.scalar.activation(out=z_t, in_=z_ps, func=COPY)
                        if not mix_b_zero:
                            nc.vector.tensor_add(z_t, z_t, mixb_bc)
                        # LayerNorm (no affine)
                        stats = swork.tile([128, 2, 6], F32, tag="stats")
                        for sg in range(2):
                            nc.vector.bn_stats(
                                out=stats[:, sg, :],
                                in_=z_t[:, sg * 512 : (sg + 1) * 512],
                            )
                        mv = swork.tile([128, 2], F32, tag="mv")
                        nc.vector.bn_aggr(out=mv, in_=stats)
                        rstd = swork.tile([128, 1], F32, tag="rstd")
                        nc.scalar.activation(
                            out=rstd,
                            in_=mv[:, 1:2],
                            func=mybir.ActivationFunctionType.Sqrt,
                            bias=eps_t,
                        )
                        nc.vector.reciprocal(out=rstd, in_=rstd)
                        nc.vector.tensor_scalar(
                            out=z_t,
                            in0=z_t,
                            scalar1=mv[:, 0:1],
                            scalar2=rstd,
                            op0=mybir.AluOpType.subtract,
                            op1=mybir.AluOpType.mult,
                        )
                        thru_t = swork.tile([128, OUT], F32, tag="thrut")
                        og_t = swork.tile([128, OUT], F32, tag="ogt")
                        nc.sync.dma_start(out=thru_t, in_=thru_d[tg : tg + 128, :])
                        nc.sync.dma_start(out=og_t, in_=og_d[tg : tg + 128, :])
                        nc.vector.tensor_sub(z_t, z_t, thru_t)
                        nc.gpsimd.tensor_mul(z_t, z_t, og_t)
                        nc.vector.tensor_add(z_t, z_t, thru_t)
                        nc.sync.dma_start(out=out_d[tg : tg + 128, :], in_=z_t)
                    # ---- s output for this block (two half-staging groups) ----
                    for h in range(2):
                        s_sb = sstage.tile([128, 2, MC, 2], F32, tag="ssb")
                        for q in range(8):
                            for comp in range(2):
                                stp = sps.tile([128, 256], F32R, tag="stp")
                                for g in range(2):
                                    nc.tensor.transpose(
                                        stp[:, g * 128 : (g + 1) * 128],
                                        srsi[q][comp][
                                            :, (2 * h + g) * 128 : (2 * h + g + 1) * 128
                                        ],
                                        ident_r,
                                    )
                                nc.scalar.activation(
                                    out=s_sb[:, :, q * 128 : (q + 1) * 128, comp],
                                    in_=stp.rearrange("p (g n) -> p g n", g=2),
                                    func=COPY,
                                )
                        nc.sync.dma_start(
                            out=bass.AP(
                                tensor=s_d,
                                offset=(t0 + h * 256) * 2 * MC,
                                ap=[[2 * MC, 128], [128 * 2 * MC, 2], [1, 2 * MC]],
                            ),
                            in_=s_sb,
                        )
    nc.compile()
    return nc


def _tables(a: np.ndarray, b: np.ndarray):
    t_abs = np.arange(T, dtype=np.float64)
    bb = b.astype(np.float64)  # [C]
    c_of_p = np.arange(128) % C
    ang = bb[c_of_p][:, None] * t_abs[None, :]  # [128, T]
    cosT = np.cos(ang).astype(np.float32)
    sinnT = (-np.sin(ang)).astype(np.float32)
    decay = np.exp(-np.abs(a.astype(np.float64)))  # [M]
    p = np.arange(128)
    q = np.arange(8)
    dec_col = decay[(128 * q[None, :] + p[:, None]) // C].astype(np.float32)
    return cosT, sinnT, dec_col


def _in_maps(x, pre_w, pre_b, mix_w, mix_b, a, b):
    cosT, sinnT, dec_col = _tables(a, b)
    ident = np.eye(128, dtype=np.float32)
    p = np.arange(128)
    k = np.arange(32)
    replw = np.zeros((8, 32, 128), np.float32)
    for q in range(8):
        replw[q] = (k[:, None] == (4 * q + p[None, :] // 32)).astype(np.float32)
    return [
        dict(
            xT=np.ascontiguousarray(x[core].T),
            pre_w=pre_w,
            pre_b=pre_b,
            mix_w=mix_w,
            mix_b=mix_b,
            cosT=cosT,
            sinnT=sinnT,
            decay_col=dec_col,
            ident_r=ident,
            repl_w=replw,
        )
        for core in range(B)
    ]


def kernel(x, pre_w, pre_b, mix_w, mix_b, a, b):
    global _compiled
    x = np.ascontiguousarray(np.asarray(x, np.float32))
    pre_w = np.ascontiguousarray(np.asarray(pre_w, np.float32))
    pre_b = np.ascontiguousarray(np.asarray(pre_b, np.float32))
    mix_w = np.ascontiguousarray(np.asarray(mix_w, np.float32))
    mix_b = np.ascontiguousarray(np.asarray(mix_b, np.float32))
    a = np.asarray(a, np.float32)
    b = np.asarray(b, np.float32)

    pre_b_zero = not np.any(pre_b)
    mix_b_zero = not np.any(mix_b)
    key = (pre_b_zero, mix_b_zero)
    if _compiled is None or _compiled[0] != key:
        try:
            nc = _build(pre_b_zero, mix_b_zero, srsi_bufs=2)
        except ValueError:
            nc = _build(pre_b_zero, mix_b_zero, srsi_bufs=1)
        _compiled = (key, nc)
    nc = _compiled[1]

    res = run_bass_kernel_spmd(
        nc, _in_maps(x, pre_w, pre_b, mix_w, mix_b, a, b), list(range(B))
    )
    out = np.stack([res.results[i]["out"] for i in range(B)])
    s = np.stack(
        [
            np.ascontiguousarray(res.results[i]["s_out"])
            .view(np.complex64)
            .reshape(T, M, C)
            for i in range(B)
        ]
    )
    return out, s


# revision 27
# speedup vs baseline: 1.0885x; 1.0885x over previous
"""Trainium2 Bass kernel for nn_FFM (FFM layer: pre-matmul + gated complex-decay
scan + mix-matmul + LayerNorm + gated residual).

Sharding: data-parallel over B=8 across the 8 NeuronCores (one batch element
per core). Weights/tables replicated.

Per-core algorithm (T=2048, IN=1024, OUT=1024, M=C=32):
  pre = x @ pre_w (+pre_b)  ->  y|thru|gate ; gate=sigmoid
  u = y * in_gate
  Demodulated scan (w_t = decay_m * w_{t-1} + e^{-i b_c t} u_t) computed as two
  REAL first-order scans via the DVE tensor_tensor_scan instruction, with
  (m,c) on partitions and t on the free axis; s_t = e^{+i b_c t} * w_t.
  z = interleave(re,im)(s) @ mix_w (+mix_b);  LayerNorm(z);
  out = z*out_gate + thru*(1-out_gate)
"""

import numpy as np

import concourse.bass as bass
import concourse.tile as tile
from concourse import bacc, mybir
from concourse.bass_utils import run_bass_kernel_spmd

B, T, IN, OUT, M, C = 8, 2048, 1024, 1024, 32, 32
J = 2 * M + 2 * OUT  # 2112
MC = M * C  # 1024
EPS = 1e-5
TB = 512  # t-block for scan/mix phase
NBLK = T // TB  # 4
NTT = T // 128  # 16 t-tiles
F32 = mybir.dt.float32
F32R = mybir.dt.float32r
COPY = mybir.ActivationFunctionType.Copy

_compiled = None


def _build(pre_b_zero: bool, mix_b_zero: bool, srsi_bufs: int = 2):
    nc = bacc.Bacc("TRN2", target_bir_lowering=False, debug=False, num_devices=8)

    # ---- DRAM I/O ----
    xT_d = nc.dram_tensor("xT", [IN, T], F32R, kind="ExternalInput")
    pre_w_d = nc.dram_tensor("pre_w", [IN, J], F32R, kind="ExternalInput")
    pre_b_d = nc.dram_tensor("pre_b", [J], F32, kind="ExternalInput")
    mix_w_d = nc.dram_tensor("mix_w", [2 * MC, OUT], F32R, kind="ExternalInput")
    mix_b_d = nc.dram_tensor("mix_b", [OUT], F32, kind="ExternalInput")
    cos_d = nc.dram_tensor("cosT", [128, T], F32, kind="ExternalInput")
    sinn_d = nc.dram_tensor("sinnT", [128, T], F32, kind="ExternalInput")
    dec_d = nc.dram_tensor("decay_col", [128, 8], F32, kind="ExternalInput")
    identr_d = nc.dram_tensor("ident_r", [128, 128], F32R, kind="ExternalInput")
    replw_d = nc.dram_tensor("repl_w", [8, 32, 128], F32R, kind="ExternalInput")
    out_d = nc.dram_tensor("out", [T, OUT], F32, kind="ExternalOutput")
    s_d = nc.dram_tensor("s_out", [T, 2 * MC], F32, kind="ExternalOutput")
    # scratch between phases
    thru_d = nc.dram_tensor("thru_scr", [T, OUT], F32)
    og_d = nc.dram_tensor("og_scr", [T, OUT], F32)

    with tile.TileContext(nc) as tc:
        with (
            tc.tile_pool(name="consts", bufs=1) as consts,
            tc.tile_pool(name="mixw", bufs=1) as mixw_pool,
        ):
            mix_w_sb = mixw_pool.tile([128, 8, 2, OUT], F32R)
            nc.sync.dma_start(
                out=mix_w_sb,
                in_=mix_w_d.ap().rearrange("(q p two) o -> p q two o", p=128, two=2),
            )
            dec = consts.tile([128, 8], F32)
            ident_r = consts.tile([128, 128], F32R)
            nc.sync.dma_start(out=ident_r, in_=identr_d[:, :])
            nc.sync.dma_start(out=dec, in_=dec_d[:, :])
            uT = consts.tile([32, T], F32R)  # u transposed, filled in phase P
            repl_w = consts.tile([32, 8, 128], F32R)
            nc.sync.dma_start(
                out=repl_w, in_=replw_d.ap().rearrange("q k p -> k q p")
            )
            if not pre_b_zero:
                preb_bc = consts.tile([128, J], F32)
                nc.gpsimd.dma_start(
                    out=preb_bc,
                    in_=bass.AP(tensor=pre_b_d, offset=0, ap=[[0, 128], [1, J]]),
                )
            if not mix_b_zero:
                mixb_bc = consts.tile([128, OUT], F32)
                nc.gpsimd.dma_start(
                    out=mixb_bc,
                    in_=bass.AP(tensor=mix_b_d, offset=0, ap=[[0, 128], [1, OUT]]),
                )
            eps_t = consts.tile([128, 1], F32)
            nc.vector.memset(eps_t, EPS)

            # ============ PHASE P: pre matmul + gates ============
            with (
                tc.tile_pool(name="prew", bufs=1) as prew_pool,
                tc.tile_pool(name="pin", bufs=4) as pin,
                tc.tile_pool(name="pwork", bufs=2) as pwork,
                tc.tile_pool(name="pps", bufs=2, space="PSUM") as pps,
                tc.tile_pool(name="ppre", bufs=3, space="PSUM") as ppre,
            ):
                pre_w_sb = prew_pool.tile([128, 8, J], F32R)
                nc.sync.dma_start(
                    out=pre_w_sb,
                    in_=pre_w_d.ap().rearrange("(k p) j -> p k j", p=128),
                )
                JCS = [(0, 512), (512, 512), (1024, 512), (1536, 512), (2048, 64)]
                for tt in range(NTT):
                    xT = pin.tile([128, 8, 128], F32R, tag="xT")
                    nc.sync.dma_start(
                        out=xT,
                        in_=xT_d[:, tt * 128 : (tt + 1) * 128].rearrange(
                            "(k p) t -> p k t", p=128
                        ),
                    )
                    pre_t = pwork.tile([128, J], F32, tag="pret")
                    for j0, jn in JCS:
                        pre_ps = ppre.tile([128, 512], F32, tag="preps")
                        for k in range(8):
                            nc.tensor.matmul(
                                pre_ps[:, 0:jn],
                                xT[:, k, :],
                                pre_w_sb[:, k, j0 : j0 + jn],
                                start=(k == 0),
                                stop=(k == 7),
                            )
                        nc.scalar.activation(
                            out=pre_t[:, j0 : j0 + jn], in_=pre_ps[:, 0:jn], func=COPY
                        )
                    if not pre_b_zero:
                        nc.vector.tensor_add(pre_t, pre_t, preb_bc)
                    # gates
                    g_t = pwork.tile([128, M + OUT], F32, tag="gt")
                    nc.scalar.activation(
                        out=g_t,
                        in_=pre_t[:, M + OUT :],
                        func=mybir.ActivationFunctionType.Sigmoid,
                    )
                    u_t = pwork.tile([128, M], F32R, tag="ut")
                    nc.vector.tensor_mul(u_t, pre_t[:, 0:M], g_t[:, 0:M])
                    # transpose u -> uT[:, tt*128:]
                    up = pps.tile([32, 128], F32R, tag="xtp")
                    nc.tensor.transpose(up, u_t, ident_r)
                    nc.scalar.activation(
                        out=uT[:, tt * 128 : (tt + 1) * 128], in_=up, func=COPY
                    )
                    # spill thru / out_gate
                    nc.sync.dma_start(
                        out=thru_d[tt * 128 : (tt + 1) * 128, :],
                        in_=pre_t[:, M : M + OUT],
                    )
                    nc.sync.dma_start(
                        out=og_d[tt * 128 : (tt + 1) * 128, :], in_=g_t[:, M:]
                    )

            # ============ PHASE S: scan + mix + LN + gating ============
            with (
                tc.tile_pool(name="tabs", bufs=2) as tabs,
                tc.tile_pool(name="srsi", bufs=srsi_bufs) as srsi_pool,
                tc.tile_pool(name="swork", bufs=2) as swork,
                tc.tile_pool(name="wcar", bufs=2) as wcar,
                tc.tile_pool(name="sstage", bufs=2) as sstage,
                tc.tile_pool(name="zps", bufs=2, space="PSUM") as zps,
                tc.tile_pool(name="sps", bufs=2, space="PSUM") as sps,
            ):
                wr_prev = [None] * 8
                wi_prev = [None] * 8
                for blk in range(NBLK):
                    t0 = blk * TB
                    cos_b = tabs.tile([128, TB], F32, tag="cosb")
                    sinn_b = tabs.tile([128, TB], F32, tag="sinnb")
                    nc.sync.dma_start(out=cos_b, in_=cos_d[:, t0 : t0 + TB])
                    nc.sync.dma_start(out=sinn_b, in_=sinn_d[:, t0 : t0 + TB])
                    srsi = []
                    for q in range(8):
                        repl = sps.tile([128, TB], F32, tag="repl")
                        nc.tensor.matmul(
                            repl,
                            repl_w[:, q, :],
                            uT[:, t0 : t0 + TB],
                            start=True,
                            stop=True,
                        )
                        uc = swork.tile([128, TB], F32, tag="uc")
                        us = swork.tile([128, TB], F32, tag="us")
                        nc.vector.tensor_mul(uc, repl, cos_b)
                        nc.vector.tensor_mul(us, repl, sinn_b)
                        wr = swork.tile([128, TB], F32, tag="wr")
                        wi = swork.tile([128, TB], F32, tag="wi")
                        dq = dec[:, q : q + 1].to_broadcast((128, TB))
                        nc.vector.tensor_tensor_scan(
                            out=wr,
                            data0=dq,
                            data1=uc,
                            initial=0.0 if blk == 0 else wr_prev[q][:, 0:1],
                            op0=mybir.AluOpType.mult,
                            op1=mybir.AluOpType.add,
                        )
                        nc.vector.tensor_tensor_scan(
                            out=wi,
                            data0=dq,
                            data1=us,
                            initial=0.0 if blk == 0 else wi_prev[q][:, 0:1],
                            op0=mybir.AluOpType.mult,
                            op1=mybir.AluOpType.add,
                        )
                        if blk < NBLK - 1:
                            wrc = wcar.tile([128, 1], F32, tag=f"wrc{q}")
                            wic = wcar.tile([128, 1], F32, tag=f"wic{q}")
                            nc.scalar.activation(
                                out=wrc, in_=wr[:, TB - 1 : TB], func=COPY
                            )
                            nc.scalar.activation(
                                out=wic, in_=wi[:, TB - 1 : TB], func=COPY
                            )
                            wr_prev[q], wi_prev[q] = wrc, wic
                        # remodulate: sr = cos*wr + sinn*wi ; si = cos*wi - sinn*wr
                        m4 = swork.tile([128, TB], F32, tag="m4")
                        m2 = swork.tile([128, TB], F32, tag="m2")
                        sr = srsi_pool.tile([128, TB], F32R, tag=f"sr{q}")
                        si = srsi_pool.tile([128, TB], F32R, tag=f"si{q}")
                        nc.gpsimd.tensor_mul(m4, sinn_b, wi)
                        nc.vector.tensor_mul(m2, sinn_b, wr)
                        nc.vector.tensor_mul(uc, cos_b, wr)
                        nc.gpsimd.tensor_mul(us, cos_b, wi)
                        nc.vector.tensor_add(sr, uc, m4)
                        nc.gpsimd.tensor_sub(si, us, m2)
                        srsi.append((sr, si))
                    # ---- mix matmul + LN + gating + s-out per t-sub ----
                    for ts in range(TB // 128):
                        tg = t0 + ts * 128
                        z_ps = zps.tile([128, OUT], F32, tag="zps")
                        for kt in range(16):
                            qq, comp = kt // 2, kt % 2
                            lhs = srsi[qq][comp][:, ts * 128 : (ts + 1) * 128]
                            for oc in range(2):
                                nc.tensor.matmul(
                                    z_ps[:, oc * 512 : (oc + 1) * 512],
                                    lhs,
                                    mix_w_sb[:, qq, comp, oc * 512 : (oc + 1) * 512],
                                    start=(kt == 0),
                                    stop=(kt == 15),
                                )
                        z_t = swork.tile([128, OUT], F32, tag="zt")
                        nc.scalar.activation(out=z_t, in_=z_ps, func=COPY)
                        if not mix_b_zero:
                            nc.vector.tensor_add(z_t, z_t, mixb_bc)
                        # LayerNorm (no affine)
                        stats = swork.tile([128, 2, 6], F32, tag="stats")
                        for sg in range(2):
                            nc.vector.bn_stats(
                                out=stats[:, sg, :],
                                in_=z_t[:, sg * 512 : (sg + 1) * 512],
                            )
                        mv = swork.tile([128, 2], F32, tag="mv")
                        nc.vector.bn_aggr(out=mv, in_=stats)
                        rstd = swork.tile([128, 1], F32, tag="rstd")
                        nc.scalar.activation(
                            out=rstd,
                            in_=mv[:, 1:2],
                            func=mybir.ActivationFunctionType.Sqrt,
                            bias=eps_t,
                        )
                        nc.vector.reciprocal(out=rstd, in_=rstd)
                        nc.vector.tensor_scalar(
                            out=z_t,
                            in0=z_t,
                            scalar1=mv[:, 0:1],
                            scalar2=rstd,
                            op0=mybir.AluOpType.subtract,
                            op1=mybir.AluOpType.mult,
                        )
                        thru_t = swork.tile([128, OUT], F32, tag="thrut")
                        og_t = swork.tile([128, OUT], F32, tag="ogt")
                        nc.sync.dma_start(out=thru_t, in_=thru_d[tg : tg + 128, :])
                        nc.sync.dma_start(out=og_t, in_=og_d[tg : tg + 128, :])
                        nc.vector.tensor_sub(z_t, z_t, thru_t)
                        nc.gpsimd.tensor_mul(z_t, z_t, og_t)
                        nc.vector.tensor_add(z_t, z_t, thru_t)
                        nc.sync.dma_start(out=out_d[tg : tg + 128, :], in_=z_t)
                    # ---- s output for this block (two half-staging groups) ----
                    for h in range(2):
                        s_sb = sstage.tile([128, 2, MC, 2], F32, tag="ssb")
                        for q in range(8):
                            for comp in range(2):
                                stp = sps.tile([128, 256], F32R, tag="stp")
                                for g in range(2):
                                    nc.tensor.transpose(
                                        stp[:, g * 128 : (g + 1) * 128],
                                        srsi[q][comp][
                                            :, (2 * h + g) * 128 : (2 * h + g + 1) * 128
                                        ],
                                        ident_r,
                                    )
                                nc.scalar.activation(
                                    out=s_sb[:, :, q * 128 : (q + 1) * 128, comp],
                                    in_=stp.rearrange("p (g n) -> p g n", g=2),
                                    func=COPY,
                                )
                        nc.sync.dma_start(
                            out=bass.AP(
                                tensor=s_d,
                                offset=(t0 + h * 256) * 2 * MC,
                                ap=[[2 * MC, 128], [128 * 2 * MC, 2], [1, 2 * MC]],
                            ),
                            in_=s_sb,
                        )
    nc.compile()
    return nc


def _tables(a: np.ndarray, b: np.ndarray):
    t_abs = np.arange(T, dtype=np.float64)
    bb = b.astype(np.float64)  # [C]
    c_of_p = np.arange(128) % C
    ang = bb[c_of_p][:, None] * t_abs[None, :]  # [128, T]
    cosT = np.cos(ang).astype(np.float32)
    sinnT = (-np.sin(ang)).astype(np.float32)
    decay = np.exp(-np.abs(a.astype(np.float64)))  # [M]
    p = np.arange(128)
    q = np.arange(8)
    dec_col = decay[(128 * q[None, :] + p[:, None]) // C].astype(np.float32)
    return cosT, sinnT, dec_col


def _in_maps(x, pre_w, pre_b, mix_w, mix_b, a, b):
    cosT, sinnT, dec_col = _tables(a, b)
    ident = np.eye(128, dtype=np.float32)
    p = np.arange(128)
    k = np.arange(32)
    replw = np.zeros((8, 32, 128), np.float32)
    for q in range(8):
        replw[q] = (k[:, None] == (4 * q + p[None, :] // 32)).astype(np.float32)
    return [
        dict(
            xT=np.ascontiguousarray(x[core].T),
            pre_w=pre_w,
            pre_b=pre_b,
            mix_w=mix_w,
            mix_b=mix_b,
            cosT=cosT,
            sinnT=sinnT,
            decay_col=dec_col,
            ident_r=ident,
            repl_w=replw,
        )
        for core in range(B)
    ]


def kernel(x, pre_w, pre_b, mix_w, mix_b, a, b):
    global _compiled
    x = np.ascontiguousarray(np.asarray(x, np.float32))
    pre_w = np.ascontiguousarray(np.asarray(pre_w, np.float32))
    pre_b = np.ascontiguousarray(np.asarray(pre_b, np.float32))
    mix_w = np.ascontiguousarray(np.asarray(mix_w, np.float32))
    mix_b = np.ascontiguousarray(np.asarray(mix_b, np.float32))
    a = np.asarray(a, np.float32)
    b = np.asarray(b, np.float32)

    pre_b_zero = not np.any(pre_b)
    mix_b_zero = not np.any(mix_b)
    key = (pre_b_zero, mix_b_zero)
    if _compiled is None or _compiled[0] != key:
        nc = None
        for nbufs in (3, 2, 1):
            try:
                nc = _build(pre_b_zero, mix_b_zero, srsi_bufs=nbufs)
                break
            except ValueError:
                continue
        _compiled = (key, nc)
    nc = _compiled[1]

    res = run_bass_kernel_spmd(
        nc, _in_maps(x, pre_w, pre_b, mix_w, mix_b, a, b), list(range(B))
    )
    out = np.stack([res.results[i]["out"] for i in range(B)])
    s = np.stack(
        [
            np.ascontiguousarray(res.results[i]["s_out"])
            .view(np.complex64)
            .reshape(T, M, C)
            for i in range(B)
        ]
    )
    return out, s
